# revision 44
# baseline (speedup 1.0000x reference)
"""ChunkAwareAttention Trainium2 kernel.

Model (hardcoded): B=4, T=2048, D=512, H=8, DK=64, CHUNK=64, EPS=1e-5.
  xn = LayerNorm(x) * ln_w + ln_b
  q/k/v = heads(xn @ W{q,k,v} + b)        [B,H,T,DK]
  pk    = heads(pos_enc @ Wpos)[0]        [H,T,DK]
  scores = (q @ (k + pk)^T) / sqrt(DK)    (pos term folded into k)
  chunk-causal mask (attend own chunk + all previous), softmax, @v,
  out = concat_heads @ Wout + bout

Sharding over 8 cores: core c -> batch b = c//2, head-group g = c%2
(4 heads = 256 features per core). Each core computes a partial
(its heads' contribution) of out[b] @ Wout; host sums the two
partials per batch and adds bout.

Device-side layout: everything feature-on-partition ("transposed"):
  xnT [512, T], qT/kT [256, T], V natural [T, 256+ones-cols].
  Scores are computed transposed sT[k, q] = kT.T-block @ qT so softmax
  normalization is a ones-column in the attn@V matmul (row 64 of the
  PSUM accumulator = the softmax denominator). Scores are O(1) here
  (LN'd activations x 0.02-scaled weights), so exp without
  max-subtraction is exact in fp32.
  ln_w/ln_b are folded into the projection weights/biases on the host
  (exact rewrite: xn_aff @ W + b == xn_raw @ (ln_w[:,None]*W) + (ln_b@W + b)).
"""

import sys

if "/opt/trn_rl_repo" not in sys.path:
    sys.path.insert(0, "/opt/trn_rl_repo")

import math
import numpy as np

import concourse.bass as bass
import concourse.tile as tile
from concourse import bacc, mybir
from concourse.bass_utils import run_bass_kernel_spmd
from concourse.masks import make_identity

B, T, D, H = 4, 2048, 512, 8
DK = D // H
CHUNK = 64
EPS = 1e-5
NCORES = 8
HPC = H // 2          # heads per core = 4
F = HPC * DK          # features per core = 256
KD = D // 128         # contraction tiles over D = 4
NT = T // 128         # 128-row tiles over T = 16
F32 = mybir.dt.float32
F32R = mybir.dt.float32r
BF16 = mybir.dt.bfloat16
FP = mybir.dt.np  # noqa


USE_F32R = True


def _r(ap):
    """bitcast an AP to float32r for full-rate PE matmuls."""
    if not USE_F32R:
        return ap
    return ap.bitcast(F32R)


def _build_program():
    nc = bacc.Bacc(
        "TRN2",
        target_bir_lowering=False,
        debug=False,
        enable_asserts=False,
        num_devices=NCORES,
    )

    x_d = nc.dram_tensor("x", [T, D], F32, kind="ExternalInput").ap()
    posT_d = nc.dram_tensor("posT", [D, T], F32R, kind="ExternalInput").ap()
    wq_d = nc.dram_tensor("wq", [D, F], F32R, kind="ExternalInput").ap()
    wk_d = nc.dram_tensor("wk", [D, F], F32R, kind="ExternalInput").ap()
    wv_d = nc.dram_tensor("wv", [D, F], F32R, kind="ExternalInput").ap()
    wpos_d = nc.dram_tensor("wpos", [D, F], F32R, kind="ExternalInput").ap()
    wout_d = nc.dram_tensor("wout", [F, D], F32R, kind="ExternalInput").ap()
    bq_d = nc.dram_tensor("bq", [F, 1], F32, kind="ExternalInput").ap()
    bk_d = nc.dram_tensor("bk", [F, 1], F32, kind="ExternalInput").ap()
    bv_d = nc.dram_tensor("bv", [1, F], F32, kind="ExternalInput").ap()
    out_d = nc.dram_tensor("out", [T, D], F32, kind="ExternalOutput").ap()

    with tile.TileContext(nc) as tc:
        _emit(nc, tc, x_d, posT_d, wq_d, wk_d, wv_d, wpos_d, wout_d,
              bq_d, bk_d, bv_d, out_d)

    nc.compile()
    return nc


def _emit(nc, tc, x_d, posT_d, wq_d, wk_d, wv_d, wpos_d, wout_d,
          bq_d, bk_d, bv_d, out_d):
    from contextlib import ExitStack

    ctx = ExitStack()
    with ctx:
        singles = ctx.enter_context(tc.tile_pool(name="singles", bufs=1))
        xpool = ctx.enter_context(tc.tile_pool(name="x", bufs=6))
        xnpool = ctx.enter_context(tc.tile_pool(name="xn", bufs=3))
        pospool = ctx.enter_context(tc.tile_pool(name="pos", bufs=2))
        stats = ctx.enter_context(tc.tile_pool(name="stats", bufs=4))
        exppool = ctx.enter_context(tc.tile_pool(name="exp", bufs=6))
        rcpool = ctx.enter_context(tc.tile_pool(name="rc", bufs=1))
        opool = ctx.enter_context(tc.tile_pool(name="ostage", bufs=2))
        # PSUM pools: ps (scores/projections, [128,1024] max = 2 banks, x2)
        # + out_ps (attention accumulator, 4 banks) = 16KB/partition exactly.
        ps = ctx.enter_context(tc.tile_pool(name="ps", bufs=3, space="PSUM"))
        out_ps_pool = ctx.enter_context(
            tc.tile_pool(name="outps", bufs=1, space="PSUM"))  # 2 tags x 1

        # prefetch all x tiles FIRST so LN isn't stuck behind weight DMAs
        xts = []
        for ti in range(NT):
            xt = xpool.tile([128, D], F32, tag="x", name=f"x{ti}")
            nc.sync.dma_start(out=xt[:], in_=x_d[ti * 128:(ti + 1) * 128, :])
            xts.append(xt)

        ident = singles.tile([128, 128], F32)
        make_identity(nc, ident)

        eps_t = singles.tile([128, 1], F32)
        nc.vector.memset(eps_t, EPS)

        ones_const = singles.tile([128, F], F32)
        nc.vector.memset(ones_const, 1.0)
        zeros_const = singles.tile([128, DK], F32)
        nc.vector.memset(zeros_const, 0.0)
        zeros_const_r = singles.tile([128, DK], F32R)
        nc.vector.tensor_copy(out=zeros_const_r[:], in_=zeros_const[:])

        # ---- resident weights ----
        wq_sb = []
        wk_sb = []
        wv_sb = []
        wpos_sb = []
        for kd in range(KD):
            for (lst, dram) in ((wq_sb, wq_d), (wk_sb, wk_d),
                                (wv_sb, wv_d), (wpos_sb, wpos_d)):
                t = singles.tile([128, F], F32R, tag=f"w{len(lst)}_{id(dram)}", name=f"w{len(lst)}_{id(dram)}")
                nc.sync.dma_start(out=t[:], in_=dram[kd * 128:(kd + 1) * 128, :])
                lst.append(t)
        wout_sb = []
        for m in range(2):
            t = singles.tile([128, D], F32R, tag=f"wout{m}", name=f"wout{m}")
            nc.sync.dma_start(out=t[:], in_=wout_d[m * 128:(m + 1) * 128, :])
            wout_sb.append(t)
        bq_sb = []
        bk_sb = []
        for m in range(2):
            tq = singles.tile([128, 1], F32, tag=f"bq{m}")
            nc.sync.dma_start(out=tq[:], in_=bq_d[m * 128:(m + 1) * 128, :])
            bq_sb.append(tq)
            tk = singles.tile([128, 1], F32, tag=f"bk{m}")
            nc.sync.dma_start(out=tk[:], in_=bk_d[m * 128:(m + 1) * 128, :])
            bk_sb.append(tk)
        # bv broadcast to all 128 partitions at load time (DMA can 0-step)
        bv_sb = singles.tile([128, F], F32)
        nc.gpsimd.dma_start(
            out=bv_sb[:],
            in_=bass.AP(tensor=bv_d.tensor, offset=bv_d.offset,
                        ap=[[0, 128], [1, F]]))

        # ---- big resident activations ----
        xnT = [singles.tile([128, T], F32, tag=f"xnT{kd}", name=f"xnT{kd}") for kd in range(KD)]
        qT = [singles.tile([128, T], BF16, tag=f"qT{m}", name=f"qT{m}") for m in range(2)]
        kT = [singles.tile([128, T], BF16, tag=f"kT{m}", name=f"kT{m}") for m in range(2)]
        # V natural layout, per head [V_h(64) | ones(64)]: the 64
        # replicated ones columns make attn@V also produce the softmax
        # denominator replicated on PSUM rows 64:128 (matmul cost is
        # moving-dim driven, so the extra M columns are free).
        v_sb = [singles.tile([128, HPC * (2 * DK)], BF16, tag=f"v{ti}", name=f"v{ti}")
                for ti in range(NT)]
        # attention output (transposed, feature-on-partition), pre/post norm
        att = [singles.tile([128, T], F32, tag=f"att{m}", name=f"att{m}") for m in range(2)]

        # ====== Phase 1: LayerNorm + transpose ======
        for ti in range(NT):
            xt = xts[ti]
            st = stats.tile([128, 6], F32)
            nc.vector.bn_stats(out=st[:], in_=xt[:])
            mv = stats.tile([128, 2], F32)
            nc.vector.bn_aggr(out=mv[:], in_=st[:])
            rstd = stats.tile([128, 1], F32)
            nc.scalar.activation(
                out=rstd[:], in_=mv[:, 1:2],
                func=mybir.ActivationFunctionType.Sqrt,
                bias=eps_t[:], scale=1.0)
            nc.vector.reciprocal(out=rstd[:], in_=rstd[:])
            xnt = xnpool.tile([128, D], F32)
            nc.vector.tensor_scalar(
                out=xnt[:], in0=xt[:],
                scalar1=mv[:, 0:1], scalar2=rstd[:],
                op0=mybir.AluOpType.subtract, op1=mybir.AluOpType.mult)
            for kd in range(KD):
                pt = ps.tile([128, 128], F32, tag="ps")
                nc.tensor.transpose(
                    pt[:], xnt[:, kd * 128:(kd + 1) * 128], ident[:])
                nc.scalar.activation(
                    out=_r(xnT[kd][:, ti * 128:(ti + 1) * 128]), in_=pt[:],
                    func=mybir.ActivationFunctionType.Copy, scale=1.0)

        # ====== Phase 2: projections ======
        for tcn in range(T // 512):
            tsl = slice(tcn * 512, (tcn + 1) * 512)
            post = []
            for kd in range(KD):
                pt = pospool.tile([128, 512], F32R, tag=f"pos{kd}",
                                  name=f"pos{kd}")
                nc.sync.dma_start(out=pt[:],
                                  in_=posT_d[kd * 128:(kd + 1) * 128, tsl])
                post.append(pt)
            for m in range(2):
                msl = slice(m * 128, (m + 1) * 128)
                pq = ps.tile([128, 512], F32, tag="ps")
                for kd in range(KD):
                    nc.tensor.matmul(
                        pq[:], _r(wq_sb[kd][:, msl]), _r(xnT[kd][:, tsl]),
                        start=(kd == 0), stop=(kd == KD - 1))
                nc.vector.tensor_scalar_add(
                    out=qT[m][:, tsl], in0=pq[:], scalar1=bq_sb[m][:])
                pk = ps.tile([128, 512], F32, tag="ps")
                for kd in range(KD):
                    nc.tensor.matmul(
                        pk[:], _r(wk_sb[kd][:, msl]), _r(xnT[kd][:, tsl]),
                        start=(kd == 0), stop=False)
                for kd in range(KD):
                    nc.tensor.matmul(
                        pk[:], _r(wpos_sb[kd][:, msl]), _r(post[kd][:]),
                        start=False, stop=(kd == KD - 1))
                nc.vector.tensor_scalar_add(
                    out=kT[m][:, tsl], in0=pk[:], scalar1=bk_sb[m][:])

        # V (natural layout) + ones columns
        for ti in range(NT):
            pv = ps.tile([128, F], F32, tag="ps")
            for kd in range(KD):
                nc.tensor.matmul(
                    pv[:], _r(xnT[kd][:, ti * 128:(ti + 1) * 128]),
                    _r(wv_sb[kd][:]),
                    start=(kd == 0), stop=(kd == KD - 1))
            vt = v_sb[ti]
            dst = vt[:].rearrange("p (h c) -> p h c", h=HPC)[:, :, 0:DK]
            srcv = pv[:].rearrange("p (h c) -> p h c", c=DK)
            bvb = bv_sb[:].rearrange("p (h c) -> p h c", c=DK)
            nc.vector.tensor_tensor(
                out=dst, in0=srcv, in1=bvb, op=mybir.AluOpType.add)
            ones = vt[:].rearrange("p (h c) -> p h c", h=HPC)[:, :, DK:2 * DK]
            nc.vector.tensor_copy(
                out=ones,
                in_=ones_const[:].rearrange("p (h c) -> p h c", c=DK))

        # ================= Phase 3: attention =================
        # q-blocked: per (head, 1024-wide q block), accumulate over k-tiles.
        # out_acc rows 0:64 = unnormalized output, rows 64:128 = softmax
        # denominator (from the 64 replicated ones columns in v_sb).
        SCALE = 1.0 / math.sqrt(DK)

        def emit_out_proj(ti):
            po = ps.tile([128, D], F32, tag="ps")
            for m in range(2):
                nc.tensor.matmul(
                    po[:], _r(att[m][:, ti * 128:(ti + 1) * 128]),
                    _r(wout_sb[m][:]),
                    start=(m == 0), stop=(m == 1))
            og = opool.tile([128, D], F32, tag="ostage", name="ostage")
            if ti % 2 == 0:
                nc.vector.tensor_copy(out=og[:], in_=po[:])
            else:
                nc.scalar.activation(
                    out=og[:], in_=po[:],
                    func=mybir.ActivationFunctionType.Copy, scale=1.0)
            nc.sync.dma_start(
                out=out_d[ti * 128:(ti + 1) * 128, :], in_=og[:])

        for h in range(HPC):
            m = h // 2
            r0 = 64 * (h % 2)
            for qj in range(T // 1024):
                g = qj * 1024
                kmax = 8 * qj + 8
                out_acc = out_ps_pool.tile([128, 1024], F32, tag="oacc",
                                           name="out_acc")
                LAG = 2

                def emit_scores(ki):
                    qoff = 128 * ki
                    kst = kT[m][r0:r0 + DK, qoff:qoff + 128]
                    cs = max(qoff, g)
                    ce = g + 1024
                    spt = ps.tile([128, 1024], F32, tag="ps", name="spt")
                    for bb in range(cs // 512, (ce - 1) // 512 + 1):
                        s5 = max(cs, bb * 512)
                        e5 = min(ce, (bb + 1) * 512)
                        nc.tensor.matmul(
                            spt[:, s5 - g:e5 - g],
                            kst, qT[m][r0:r0 + DK, s5:e5],
                            start=True, stop=True)
                    et = exppool.tile([128, 1024], BF16, tag="et", name="et")
                    nc.scalar.activation(
                        out=et[:, cs - g:1024], in_=spt[:, cs - g:1024],
                        func=mybir.ActivationFunctionType.Exp,
                        scale=SCALE)
                    if cs == qoff:
                        # mask keys of chunk 2ki+1 vs queries of chunk 2ki
                        nc.vector.tensor_copy(
                            out=et[64:128, cs - g:cs - g + 64],
                            in_=zeros_const[0:DK, :])
                    return et

                def emit_attnv(ki, et):
                    qoff = 128 * ki
                    vst = v_sb[ki][:, (h % 4) * 2 * DK:((h % 4) + 1) * 2 * DK]
                    cs = max(qoff, g)
                    ce = g + 1024
                    for bb in range(cs // 512, (ce - 1) // 512 + 1):
                        s5 = max(cs, bb * 512)
                        e5 = min(ce, (bb + 1) * 512)
                        last_ki = min(8 * qj + 4 * (bb - 2 * qj) + 3, kmax - 1)
                        nc.tensor.matmul(
                            out_acc[:, s5 - g:e5 - g],
                            vst, et[:, s5 - g:e5 - g],
                            start=(ki == 0), stop=(ki == last_ki))

                pend = []
                for ki in range(kmax):
                    pend.append((ki, emit_scores(ki)))
                    if len(pend) > LAG:
                        k0, e0 = pend.pop(0)
                        emit_attnv(k0, e0)
                for k0, e0 in pend:
                    emit_attnv(k0, e0)
                # normalize + evict this q block
                dn = rcpool.tile([DK, 1024], F32, tag="dn", name="dn")
                nc.vector.tensor_copy(out=dn[:], in_=out_acc[DK:2 * DK, :])
                rc = rcpool.tile([DK, 1024], F32, tag="rc", name="rc")
                nc.vector.reciprocal_approx_fast(out=rc[:], in_=dn[:])
                nc.vector.tensor_tensor(
                    out=_r(att[m][r0:r0 + DK, g:g + 1024]),
                    in0=out_acc[0:DK, :],
                    in1=rc[:], op=mybir.AluOpType.mult)

        # ====== Phase 4: output projection ======
        for ti in range(NT):
            emit_out_proj(ti)


# revision 45
# speedup vs baseline: 1.1422x; 1.1422x over previous
"""ChunkAwareAttention Trainium2 kernel.

Model (hardcoded): B=4, T=2048, D=512, H=8, DK=64, CHUNK=64, EPS=1e-5.
  xn = LayerNorm(x) * ln_w + ln_b
  q/k/v = heads(xn @ W{q,k,v} + b)        [B,H,T,DK]
  pk    = heads(pos_enc @ Wpos)[0]        [H,T,DK]
  scores = (q @ (k + pk)^T) / sqrt(DK)    (pos term folded into k)
  chunk-causal mask (attend own chunk + all previous), softmax, @v,
  out = concat_heads @ Wout + bout

Sharding over 8 cores: core c -> batch b = c//2, head-group g = c%2
(4 heads = 256 features per core). Each core computes a partial
(its heads' contribution) of out[b] @ Wout; host sums the two
partials per batch and adds bout.

Device-side layout: everything feature-on-partition ("transposed"):
  xnT [512, T], qT/kT [256, T], V natural [T, 256+ones-cols].
  Scores are computed transposed sT[k, q] = kT.T-block @ qT so softmax
  normalization is a ones-column in the attn@V matmul (row 64 of the
  PSUM accumulator = the softmax denominator). Scores are O(1) here
  (LN'd activations x 0.02-scaled weights), so exp without
  max-subtraction is exact in fp32.
  ln_w/ln_b are folded into the projection weights/biases on the host
  (exact rewrite: xn_aff @ W + b == xn_raw @ (ln_w[:,None]*W) + (ln_b@W + b)).
"""

import sys

if "/opt/trn_rl_repo" not in sys.path:
    sys.path.insert(0, "/opt/trn_rl_repo")

import math
import numpy as np

import concourse.bass as bass
import concourse.tile as tile
from concourse import bacc, mybir
from concourse.bass_utils import run_bass_kernel_spmd
from concourse.masks import make_identity

B, T, D, H = 4, 2048, 512, 8
DK = D // H
CHUNK = 64
EPS = 1e-5
NCORES = 8
HPC = H // 2          # heads per core = 4
F = HPC * DK          # features per core = 256
KD = D // 128         # contraction tiles over D = 4
NT = T // 128         # 128-row tiles over T = 16
F32 = mybir.dt.float32
F32R = mybir.dt.float32r
BF16 = mybir.dt.bfloat16
FP = mybir.dt.np  # noqa


USE_F32R = True


def _r(ap):
    """bitcast an AP to float32r for full-rate PE matmuls."""
    if not USE_F32R:
        return ap
    return ap.bitcast(F32R)


def _build_program():
    nc = bacc.Bacc(
        "TRN2",
        target_bir_lowering=False,
        debug=False,
        enable_asserts=False,
        num_devices=NCORES,
    )

    x_d = nc.dram_tensor("x", [T, D], F32, kind="ExternalInput").ap()
    posT_d = nc.dram_tensor("posT", [D, T], F32R, kind="ExternalInput").ap()
    wq_d = nc.dram_tensor("wq", [D, F], F32R, kind="ExternalInput").ap()
    wk_d = nc.dram_tensor("wk", [D, F], F32R, kind="ExternalInput").ap()
    wv_d = nc.dram_tensor("wv", [D, F], F32R, kind="ExternalInput").ap()
    wpos_d = nc.dram_tensor("wpos", [D, F], F32R, kind="ExternalInput").ap()
    wout_d = nc.dram_tensor("wout", [F, D], F32R, kind="ExternalInput").ap()
    bq_d = nc.dram_tensor("bq", [F, 1], F32, kind="ExternalInput").ap()
    bk_d = nc.dram_tensor("bk", [F, 1], F32, kind="ExternalInput").ap()
    bv_d = nc.dram_tensor("bv", [1, F], F32, kind="ExternalInput").ap()
    out_d = nc.dram_tensor("out", [T, D], F32, kind="ExternalOutput").ap()

    with tile.TileContext(nc) as tc:
        _emit(nc, tc, x_d, posT_d, wq_d, wk_d, wv_d, wpos_d, wout_d,
              bq_d, bk_d, bv_d, out_d)

    nc.compile()
    return nc


def _emit(nc, tc, x_d, posT_d, wq_d, wk_d, wv_d, wpos_d, wout_d,
          bq_d, bk_d, bv_d, out_d):
    from contextlib import ExitStack

    ctx = ExitStack()
    with ctx:
        singles = ctx.enter_context(tc.tile_pool(name="singles", bufs=1))
        xpool = ctx.enter_context(tc.tile_pool(name="x", bufs=6))
        xnpool = ctx.enter_context(tc.tile_pool(name="xn", bufs=3))
        pospool = ctx.enter_context(tc.tile_pool(name="pos", bufs=2))
        stats = ctx.enter_context(tc.tile_pool(name="stats", bufs=4))
        exppool = ctx.enter_context(tc.tile_pool(name="exp", bufs=4))
        rcpool = ctx.enter_context(tc.tile_pool(name="rc", bufs=1))
        opool = ctx.enter_context(tc.tile_pool(name="ostage", bufs=2))
        # PSUM pools: ps (scores/projections, [128,1024] max = 2 banks, x2)
        # + out_ps (attention accumulator, 4 banks) = 16KB/partition exactly.
        ps = ctx.enter_context(tc.tile_pool(name="ps", bufs=3, space="PSUM"))
        out_ps_pool = ctx.enter_context(
            tc.tile_pool(name="outps", bufs=1, space="PSUM"))  # 2 tags x 1

        # prefetch all x tiles FIRST so LN isn't stuck behind weight DMAs
        xts = []
        for ti in range(NT):
            xt = xpool.tile([128, D], F32, tag="x", name=f"x{ti}")
            nc.sync.dma_start(out=xt[:], in_=x_d[ti * 128:(ti + 1) * 128, :])
            xts.append(xt)

        ident = singles.tile([128, 128], F32)
        make_identity(nc, ident)

        eps_t = singles.tile([128, 1], F32)
        nc.vector.memset(eps_t, EPS)

        ones_const = singles.tile([128, F], F32)
        nc.vector.memset(ones_const, 1.0)
        zeros_const = singles.tile([128, DK], F32)
        nc.vector.memset(zeros_const, 0.0)
        zeros_const_r = singles.tile([128, DK], F32R)
        nc.vector.tensor_copy(out=zeros_const_r[:], in_=zeros_const[:])

        # ---- resident weights ----
        wq_sb = []
        wk_sb = []
        wv_sb = []
        wpos_sb = []
        for kd in range(KD):
            for (lst, dram) in ((wq_sb, wq_d), (wk_sb, wk_d),
                                (wv_sb, wv_d), (wpos_sb, wpos_d)):
                t = singles.tile([128, F], F32R, tag=f"w{len(lst)}_{id(dram)}", name=f"w{len(lst)}_{id(dram)}")
                nc.sync.dma_start(out=t[:], in_=dram[kd * 128:(kd + 1) * 128, :])
                lst.append(t)
        wout_sb = []
        for m in range(2):
            t = singles.tile([128, D], F32R, tag=f"wout{m}", name=f"wout{m}")
            nc.sync.dma_start(out=t[:], in_=wout_d[m * 128:(m + 1) * 128, :])
            wout_sb.append(t)
        bq_sb = []
        bk_sb = []
        for m in range(2):
            tq = singles.tile([128, 1], F32, tag=f"bq{m}")
            nc.sync.dma_start(out=tq[:], in_=bq_d[m * 128:(m + 1) * 128, :])
            bq_sb.append(tq)
            tk = singles.tile([128, 1], F32, tag=f"bk{m}")
            nc.sync.dma_start(out=tk[:], in_=bk_d[m * 128:(m + 1) * 128, :])
            bk_sb.append(tk)
        # bv broadcast to all 128 partitions at load time (DMA can 0-step)
        bv_sb = singles.tile([128, F], F32)
        nc.gpsimd.dma_start(
            out=bv_sb[:],
            in_=bass.AP(tensor=bv_d.tensor, offset=bv_d.offset,
                        ap=[[0, 128], [1, F]]))

        # ---- big resident activations ----
        xnT = [singles.tile([128, T], F32, tag=f"xnT{kd}", name=f"xnT{kd}") for kd in range(KD)]
        qT = [singles.tile([128, T], BF16, tag=f"qT{m}", name=f"qT{m}") for m in range(2)]
        kT = [singles.tile([128, T], BF16, tag=f"kT{m}", name=f"kT{m}") for m in range(2)]
        # V natural layout, per head [V_h(64) | ones(64)]: the 64
        # replicated ones columns make attn@V also produce the softmax
        # denominator replicated on PSUM rows 64:128 (matmul cost is
        # moving-dim driven, so the extra M columns are free).
        v_sb = [singles.tile([128, HPC * (2 * DK)], BF16, tag=f"v{ti}", name=f"v{ti}")
                for ti in range(NT)]
        # attention output (transposed, feature-on-partition), pre/post norm
        att = [singles.tile([128, T], F32, tag=f"att{m}", name=f"att{m}") for m in range(2)]

        # ====== Phase 1: LayerNorm + transpose ======
        for ti in range(NT):
            xt = xts[ti]
            st = stats.tile([128, 6], F32)
            nc.vector.bn_stats(out=st[:], in_=xt[:])
            mv = stats.tile([128, 2], F32)
            nc.vector.bn_aggr(out=mv[:], in_=st[:])
            rstd = stats.tile([128, 1], F32)
            nc.scalar.activation(
                out=rstd[:], in_=mv[:, 1:2],
                func=mybir.ActivationFunctionType.Sqrt,
                bias=eps_t[:], scale=1.0)
            nc.vector.reciprocal(out=rstd[:], in_=rstd[:])
            xnt = xnpool.tile([128, D], F32)
            nc.vector.tensor_scalar(
                out=xnt[:], in0=xt[:],
                scalar1=mv[:, 0:1], scalar2=rstd[:],
                op0=mybir.AluOpType.subtract, op1=mybir.AluOpType.mult)
            for kd in range(KD):
                pt = ps.tile([128, 128], F32, tag="ps")
                nc.tensor.transpose(
                    pt[:], xnt[:, kd * 128:(kd + 1) * 128], ident[:])
                nc.scalar.activation(
                    out=_r(xnT[kd][:, ti * 128:(ti + 1) * 128]), in_=pt[:],
                    func=mybir.ActivationFunctionType.Copy, scale=1.0)

        # ====== Phase 2: projections ======
        for tcn in range(T // 512):
            tsl = slice(tcn * 512, (tcn + 1) * 512)
            post = []
            for kd in range(KD):
                pt = pospool.tile([128, 512], F32R, tag=f"pos{kd}",
                                  name=f"pos{kd}")
                nc.sync.dma_start(out=pt[:],
                                  in_=posT_d[kd * 128:(kd + 1) * 128, tsl])
                post.append(pt)
            for m in range(2):
                msl = slice(m * 128, (m + 1) * 128)
                pq = ps.tile([128, 512], F32, tag="ps")
                for kd in range(KD):
                    nc.tensor.matmul(
                        pq[:], _r(wq_sb[kd][:, msl]), _r(xnT[kd][:, tsl]),
                        start=(kd == 0), stop=(kd == KD - 1))
                nc.vector.tensor_scalar_add(
                    out=qT[m][:, tsl], in0=pq[:], scalar1=bq_sb[m][:])
                pk = ps.tile([128, 512], F32, tag="ps")
                for kd in range(KD):
                    nc.tensor.matmul(
                        pk[:], _r(wk_sb[kd][:, msl]), _r(xnT[kd][:, tsl]),
                        start=(kd == 0), stop=False)
                for kd in range(KD):
                    nc.tensor.matmul(
                        pk[:], _r(wpos_sb[kd][:, msl]), _r(post[kd][:]),
                        start=False, stop=(kd == KD - 1))
                nc.vector.tensor_scalar_add(
                    out=kT[m][:, tsl], in0=pk[:], scalar1=bk_sb[m][:])

        # V (natural layout) + ones columns
        for ti in range(NT):
            pv = ps.tile([128, F], F32, tag="ps")
            for kd in range(KD):
                nc.tensor.matmul(
                    pv[:], _r(xnT[kd][:, ti * 128:(ti + 1) * 128]),
                    _r(wv_sb[kd][:]),
                    start=(kd == 0), stop=(kd == KD - 1))
            vt = v_sb[ti]
            dst = vt[:].rearrange("p (h c) -> p h c", h=HPC)[:, :, 0:DK]
            srcv = pv[:].rearrange("p (h c) -> p h c", c=DK)
            bvb = bv_sb[:].rearrange("p (h c) -> p h c", c=DK)
            nc.vector.tensor_tensor(
                out=dst, in0=srcv, in1=bvb, op=mybir.AluOpType.add)
            ones = vt[:].rearrange("p (h c) -> p h c", h=HPC)[:, :, DK:2 * DK]
            nc.vector.tensor_copy(
                out=ones,
                in_=ones_const[:].rearrange("p (h c) -> p h c", c=DK))

        # ================= Phase 3: attention =================
        # q-blocked: per (head, 1024-wide q block), accumulate over k-tiles.
        # out_acc rows 0:64 = unnormalized output, rows 64:128 = softmax
        # denominator (from the 64 replicated ones columns in v_sb).
        SCALE = 1.0 / math.sqrt(DK)

        def emit_out_proj(ti):
            po = ps.tile([128, D], F32, tag="ps")
            for m in range(2):
                nc.tensor.matmul(
                    po[:], _r(att[m][:, ti * 128:(ti + 1) * 128]),
                    _r(wout_sb[m][:]),
                    start=(m == 0), stop=(m == 1))
            og = opool.tile([128, D], F32, tag="ostage", name="ostage")
            if ti % 2 == 0:
                nc.vector.tensor_copy(out=og[:], in_=po[:])
            else:
                nc.scalar.activation(
                    out=og[:], in_=po[:],
                    func=mybir.ActivationFunctionType.Copy, scale=1.0)
            nc.sync.dma_start(
                out=out_d[ti * 128:(ti + 1) * 128, :], in_=og[:])

        for h in range(HPC):
            m = h // 2
            r0 = 64 * (h % 2)
            for qj in range(T // 1024):
                g = qj * 1024
                kmax = 8 * qj + 8
                out_acc = out_ps_pool.tile([128, 1024], F32, tag="oacc",
                                           name="out_acc")
                LAG = 2

                def emit_scores(ki):
                    qoff = 128 * ki
                    kst = kT[m][r0:r0 + DK, qoff:qoff + 128]
                    cs = max(qoff, g)
                    ce = g + 1024
                    spt = ps.tile([128, 1024], F32, tag="ps", name="spt")
                    for bb in range(cs // 512, (ce - 1) // 512 + 1):
                        s5 = max(cs, bb * 512)
                        e5 = min(ce, (bb + 1) * 512)
                        nc.tensor.matmul(
                            spt[:, s5 - g:e5 - g],
                            kst, qT[m][r0:r0 + DK, s5:e5],
                            start=True, stop=True)
                    et = exppool.tile([128, 1024], BF16, tag="et", name="et")
                    nc.scalar.activation(
                        out=et[:, cs - g:1024], in_=spt[:, cs - g:1024],
                        func=mybir.ActivationFunctionType.Exp,
                        scale=SCALE)
                    if cs == qoff:
                        # mask keys of chunk 2ki+1 vs queries of chunk 2ki
                        nc.vector.tensor_copy(
                            out=et[64:128, cs - g:cs - g + 64],
                            in_=zeros_const[0:DK, :])
                    return et

                def emit_attnv(ki, et):
                    qoff = 128 * ki
                    vst = v_sb[ki][:, (h % 4) * 2 * DK:((h % 4) + 1) * 2 * DK]
                    cs = max(qoff, g)
                    ce = g + 1024
                    for bb in range(cs // 512, (ce - 1) // 512 + 1):
                        s5 = max(cs, bb * 512)
                        e5 = min(ce, (bb + 1) * 512)
                        last_ki = min(8 * qj + 4 * (bb - 2 * qj) + 3, kmax - 1)
                        nc.tensor.matmul(
                            out_acc[:, s5 - g:e5 - g],
                            vst, et[:, s5 - g:e5 - g],
                            start=(ki == 0), stop=(ki == last_ki))

                pend = []
                for ki in range(kmax):
                    pend.append((ki, emit_scores(ki)))
                    if len(pend) > LAG:
                        k0, e0 = pend.pop(0)
                        emit_attnv(k0, e0)
                for k0, e0 in pend:
                    emit_attnv(k0, e0)
                # normalize + evict this q block
                dn = rcpool.tile([DK, 1024], F32, tag="dn", name="dn")
                nc.vector.tensor_copy(out=dn[:], in_=out_acc[DK:2 * DK, :])
                rc = rcpool.tile([DK, 1024], F32, tag="rc", name="rc")
                nc.vector.reciprocal_approx_fast(out=rc[:], in_=dn[:])
                nc.vector.tensor_tensor(
                    out=_r(att[m][r0:r0 + DK, g:g + 1024]),
                    in0=out_acc[0:DK, :],
                    in1=rc[:], op=mybir.AluOpType.mult)

        # ====== Phase 4: output projection ======
        for ti in range(NT):
            emit_out_proj(ti)
